# revision 2
# baseline (speedup 1.0000x reference)
"""Trainium2 Bass kernel: ClusterActivation (nearest-centroid routing +
per-row normalization + per-cluster activation).

Data-parallel over 8 NeuronCores: x is sharded along rows, centroids are
replicated. Per core, rows are processed in [128, 512] tiles:
  - nearest centroid: PE transposes the x tile (f32, exact), then f32
    matmuls against the centroid chunks accumulate x @ c^T - |c|^2/2 in
    PSUM; argmax via DVE reduce + is_equal gives one-hot masks.
  - mean/var: DVE bn_stats/bn_aggr; inv_std = rsqrt(var) via Newton
    iterations on DVE (avoids ACT table switches; sqrt is not in the
    gelu table set).
  - activations: two ACT passes from a single LUT set
    (gelu_apprx_tanh_and_others): Gelu_apprx_tanh with per-row masked
    scale/bias, and Tanh serving both tanh rows and sigmoid rows via
    sigmoid(z) = 0.5*tanh(z/2) + 0.5. Relu is done on DVE. Per-row
    scale = mask * inv_std and bias = -mask * mean * inv_std fold the
    normalization into each activation pass.
"""

import numpy as np
from contextlib import ExitStack

import concourse.bacc as bacc
import concourse.mybir as mybir
from concourse.tile import TileContext
from concourse import bass_utils

F32 = mybir.dt.float32
AF = mybir.ActivationFunctionType
OP = mybir.AluOpType
AX = mybir.AxisListType

N_TOTAL, D, C = 100000, 512, 4
N_CORES = 8
ROWS = N_TOTAL // N_CORES  # 12500 rows per core
P = 128
KCHUNKS = D // P  # 4
EPS = 1e-5
VAR_SCALE = D / (D - 1.0)  # unbiased variance correction


def build_program(rows):
    nc = bacc.Bacc("TRN2", target_bir_lowering=False, debug=False)
    x = nc.dram_tensor("x", [rows, D], F32, kind="ExternalInput").ap()
    # ct[d, k*C + j] = centroids[j, k*128 + d]
    ct = nc.dram_tensor("ct", [P, KCHUNKS * C], F32, kind="ExternalInput").ap()
    # negn[0, j] = -|c_j|^2 / 2
    negn = nc.dram_tensor("negn", [1, C], F32, kind="ExternalInput").ap()
    ident = nc.dram_tensor("ident", [P, P], F32, kind="ExternalInput").ap()
    out = nc.dram_tensor("out", [rows, D], F32, kind="ExternalOutput").ap()

    tiles = []
    r = 0
    while r < rows:
        pr = min(P, rows - r)
        tiles.append((r, pr))
        r += pr

    with ExitStack() as ctx:
        tc = ctx.enter_context(TileContext(nc))
        cpool = ctx.enter_context(tc.tile_pool(name="const", bufs=1))
        xpool = ctx.enter_context(tc.tile_pool(name="xin", bufs=3))
        xtpool = ctx.enter_context(tc.tile_pool(name="xt", bufs=2))
        apool = ctx.enter_context(tc.tile_pool(name="acts", bufs=2))
        opool = ctx.enter_context(tc.tile_pool(name="outs", bufs=3))
        spool = ctx.enter_context(tc.tile_pool(name="small", bufs=3))
        ptpool = ctx.enter_context(tc.tile_pool(name="pxt", bufs=2, space="PSUM"))
        pspool = ctx.enter_context(tc.tile_pool(name="psc", bufs=2, space="PSUM"))

        ct_sb = cpool.tile([P, KCHUNKS, C], F32)
        nc.sync.dma_start(ct_sb[:], ct.rearrange("d (k j) -> d k j", k=KCHUNKS))
        id_sb = cpool.tile([P, P], F32)
        nc.sync.dma_start(id_sb[:], ident)
        negn_sb = cpool.tile([1, C], F32)
        nc.sync.dma_start(negn_sb[:], negn)
        ones_sb = cpool.tile([1, P], F32)
        nc.vector.memset(ones_sb[:], 1.0)

        for r0, pr in tiles:
            xt = xpool.tile([P, D], F32, tag="x")
            nc.sync.dma_start(xt[:pr], x[r0 : r0 + pr, :])
            xs = xt[:pr]

            # ---- per-row stats ----
            st6 = spool.tile([P, 6], F32, tag="st6")
            nc.vector.bn_stats(st6[:pr], xs)
            mv = spool.tile([P, 2], F32, tag="mv")
            nc.vector.bn_aggr(mv[:pr], st6[:pr])
            mean = mv[:pr, 0:1]
            var = mv[:pr, 1:2]
            vv = spool.tile([P, 1], F32, tag="vv")
            nc.vector.tensor_scalar(vv[:pr], var, VAR_SCALE, EPS, OP.mult, OP.add)
            # inv_std = rsqrt(vv): linear seed + 2 Newton steps (exact f32 ALU)
            al = spool.tile([P, 1], F32, tag="seed")
            nc.vector.tensor_scalar(al[:pr], vv[:pr], -0.5, 1.5, OP.mult, OP.add)
            for it in range(2):
                t2 = spool.tile([P, 1], F32, tag=f"nw{it}a")
                nc.vector.tensor_tensor(t2[:pr], al[:pr], al[:pr], OP.mult)
                t3 = spool.tile([P, 1], F32, tag=f"nw{it}b")
                nc.vector.tensor_tensor(t3[:pr], t2[:pr], vv[:pr], OP.mult)
                w = spool.tile([P, 1], F32, tag=f"nw{it}c")
                nc.vector.tensor_scalar(w[:pr], t3[:pr], -0.5, 1.5, OP.mult, OP.add)
                al2 = spool.tile([P, 1], F32, tag=f"nw{it}d")
                nc.vector.tensor_tensor(al2[:pr], al[:pr], w[:pr], OP.mult)
                al = al2
            alpha = al
            beta = spool.tile([P, 1], F32, tag="beta")
            nc.vector.scalar_tensor_tensor(
                beta[:pr], mean, -1.0, alpha[:pr], OP.mult, OP.mult
            )

            # ---- nearest centroid ----
            pxt = ptpool.tile([P, KCHUNKS, P], F32, tag="pxt")
            for k in range(KCHUNKS):
                nc.tensor.transpose(
                    pxt[:, k, :pr], xs[:, k * P : (k + 1) * P], id_sb[:pr, :pr]
                )
            xtsb = xtpool.tile([P, KCHUNKS, P], F32, tag="xtsb")
            nc.scalar.copy(xtsb[:, :, :pr], pxt[:, :, :pr])
            psc = pspool.tile([P, C], F32, tag="psc")
            # scores = x @ c^T - |c|^2/2  (argmax == argmin of euclidean d2)
            nc.tensor.matmul(
                psc[:pr], lhsT=ones_sb[:1, :pr], rhs=negn_sb[:], start=True, stop=False
            )
            for k in range(KCHUNKS):
                nc.tensor.matmul(
                    psc[:pr],
                    lhsT=xtsb[:, k, :pr],
                    rhs=ct_sb[:, k, :],
                    start=False,
                    stop=(k == KCHUNKS - 1),
                )
            smax = spool.tile([P, 1], F32, tag="smax")
            nc.vector.tensor_reduce(smax[:pr], psc[:pr], axis=AX.X, op=OP.max)
            masks = spool.tile([P, C], F32, tag="masks")
            nc.vector.tensor_scalar(masks[:pr], psc[:pr], smax[:pr], None, OP.is_equal)
            m_r = masks[:pr, 0:1]
            m_t = masks[:pr, 1:2]
            m_g = masks[:pr, 2:3]
            m_s = masks[:pr, 3:4]

            # ---- per-row activation coefficients ----
            # tanh pass handles tanh rows (arg xn) and sigmoid rows (arg xn/2)
            a_t = spool.tile([P, 1], F32, tag="a_t")
            nc.vector.scalar_tensor_tensor(a_t[:pr], m_s, 0.5, m_t, OP.mult, OP.add)
            s_t = spool.tile([P, 1], F32, tag="s_t")
            nc.vector.tensor_tensor(s_t[:pr], a_t[:pr], alpha[:pr], OP.mult)
            b_t = spool.tile([P, 1], F32, tag="b_t")
            nc.vector.tensor_tensor(b_t[:pr], a_t[:pr], beta[:pr], OP.mult)
            s_g = spool.tile([P, 1], F32, tag="s_g")
            nc.vector.tensor_tensor(s_g[:pr], m_g, alpha[:pr], OP.mult)
            b_g = spool.tile([P, 1], F32, tag="b_g")
            nc.vector.tensor_tensor(b_g[:pr], m_g, beta[:pr], OP.mult)
            s_r = spool.tile([P, 1], F32, tag="s_r")
            nc.vector.tensor_tensor(s_r[:pr], m_r, alpha[:pr], OP.mult)
            b_r = spool.tile([P, 1], F32, tag="b_r")
            nc.vector.tensor_tensor(b_r[:pr], m_r, beta[:pr], OP.mult)
            b2 = spool.tile([P, 1], F32, tag="b2")
            nc.vector.tensor_scalar(b2[:pr], m_s, 0.5, None, OP.mult)

            # ---- activation passes + combine ----
            pg = apool.tile([P, D], F32, tag="pg")
            nc.scalar.activation(
                pg[:pr], xs, AF.Gelu_apprx_tanh, bias=b_g[:pr], scale=s_g[:pr]
            )
            th = apool.tile([P, D], F32, tag="th")
            nc.scalar.activation(th[:pr], xs, AF.Tanh, bias=b_t[:pr], scale=s_t[:pr])
            c2 = apool.tile([P, D], F32, tag="c2")
            nc.vector.tensor_scalar(
                c2[:pr], th[:pr], a_t[:pr], b2[:pr], OP.mult, OP.add
            )
            zr = apool.tile([P, D], F32, tag="zr")
            nc.vector.tensor_scalar(
                zr[:pr], xs, s_r[:pr], b_r[:pr], OP.mult, OP.add
            )
            s1 = apool.tile([P, D], F32, tag="s1")
            nc.vector.tensor_tensor(s1[:pr], pg[:pr], c2[:pr], OP.add)
            ot = opool.tile([P, D], F32, tag="ot")
            nc.vector.scalar_tensor_tensor(
                ot[:pr], zr[:pr], 0.0, s1[:pr], OP.max, OP.add
            )
            nc.sync.dma_start(out[r0 : r0 + pr, :], ot[:pr])

    nc.compile()
    return nc


_nc_cache = {}


def _get_nc(rows):
    if rows not in _nc_cache:
        _nc_cache[rows] = build_program(rows)
    return _nc_cache[rows]


def make_const_inputs(centroids):
    c = np.asarray(centroids, dtype=np.float32)
    ct = np.empty((P, KCHUNKS * C), np.float32)
    for k in range(KCHUNKS):
        ct[:, k * C : (k + 1) * C] = c[:, k * P : (k + 1) * P].T
    negn = (-0.5 * np.sum(c.astype(np.float64) ** 2, axis=1)).astype(
        np.float32
    ).reshape(1, C)
    ident = np.eye(P, dtype=np.float32)
    return ct, negn, ident


def run_sharded(x, centroids, **spmd_kwargs):
    x = np.ascontiguousarray(np.asarray(x), dtype=np.float32)
    assert x.shape == (N_TOTAL, D)
    nc = _get_nc(ROWS)
    ct, negn, ident = make_const_inputs(centroids)
    in_maps = []
    for ci in range(N_CORES):
        in_maps.append(
            {
                "x": np.ascontiguousarray(x[ci * ROWS : (ci + 1) * ROWS]),
                "ct": ct,
                "negn": negn,
                "ident": ident,
            }
        )
    res = bass_utils.run_bass_kernel_spmd(
        nc, in_maps, core_ids=list(range(N_CORES)), **spmd_kwargs
    )
    out = np.concatenate([r["out"] for r in res.results], axis=0)
    return out, res


def kernel(x, centroids):
    out, _ = run_sharded(x, centroids)
    return out
